# revision 1
# baseline (speedup 1.0000x reference)
"""Malvar demosaic on Trainium2 (Bass/Tile), 8-core data parallel — v2.

Input path: the whole image loads via 4 strided-partition instructions
(partition p = 4t + c, disjoint 21-row grid), 8 small edge DMAs, and one
SBUF->SBUF halo copy — 13 instructions instead of 25+ (each DMA
instruction pays a ~1.6us serialized fixed cost on TRN2, so instruction
count dominates the input wall).

Software pipelining: the For_i body processes two phases — load slot A
while computing the image from slot B (loaded by the previous phase),
then swap. Input-ring drains overlap PE + output-ring of the other slot.
An epilogue after the loop computes the final load, so single-shot runs
are correct (the first in-loop compute reads a zeroed slot and its
output is overwritten later).

Layout: big tile [92, 25*514]; block b occupies free cols [514b, 514b+514)
with zero halo columns at offsets 0 and 513; partition p = 4t + c holds
input row 21b-1+t of channel c (t in [0,23)).
"""
import contextlib

import numpy as np

H, W = 512, 512
N_CORES = 8
N_ROWS = 21            # output packed rows per block
K_ROWS = N_ROWS + 2    # input rows incl halo
K_PART = 4 * K_ROWS    # 92
M_PART = 6 * N_ROWS    # 126
N_BLOCKS = (H + N_ROWS - 1) // N_ROWS  # 25
WP = W + 2             # per-block column pitch

_G_AT_R = np.array([[0,0,-1,0,0],[0,0,2,0,0],[-1,2,4,2,-1],[0,0,2,0,0],[0,0,-1,0,0]], np.float32) / 8
_R_AT_G1 = np.array([[0,0,0.5,0,0],[0,-1,0,-1,0],[-1,4,5,4,-1],[0,-1,0,-1,0],[0,0,0.5,0,0]], np.float32) / 8
_R_AT_G2 = np.array([[0,0,-1,0,0],[0,-1,4,-1,0],[0.5,0,5,0,0.5],[0,-1,4,-1,0],[0,0,-1,0,0]], np.float32) / 8
_R_AT_B = np.array([[0,0,-1.5,0,0],[0,2,0,2,0],[-1.5,0,6,0,-1.5],[0,2,0,2,0],[0,0,-1.5,0,0]], np.float32) / 8

PLANES = {
    (0, 0, 0): ('conv', _R_AT_B),
    (0, 0, 1): ('conv', _R_AT_G2),
    (0, 1, 0): ('conv', _R_AT_G1),
    (0, 1, 1): ('id', 2),
    (1, 0, 0): ('conv', _G_AT_R),
    (1, 0, 1): ('id', 0),
    (1, 1, 0): ('id', 3),
    (1, 1, 1): ('conv', _G_AT_R),
    (2, 0, 0): ('id', 1),
    (2, 0, 1): ('conv', _R_AT_G1),
    (2, 1, 0): ('conv', _R_AT_G2),
    (2, 1, 1): ('conv', _R_AT_B),
}


def _packed_weights():
    out = {}
    for (ch, r, s), (kind, val) in PLANES.items():
        Wk = np.zeros((4, 3, 3), np.float32)
        if kind == 'id':
            Wk[val, 1, 1] = 1.0
        else:
            for u in range(-2, 3):
                for v in range(-2, 3):
                    w = val[u + 2, v + 2]
                    if w == 0:
                        continue
                    rc = (r + u) % 2
                    di = (r + u - rc) // 2
                    sc = (s + v) % 2
                    dj = (s + v - sc) // 2
                    Wk[2 * rc + sc, di + 1, dj + 1] += w
        out[(ch, r, s)] = Wk
    return out


def _lhsT_matrices():
    """lhsT[s][dj] as [K_PART, M_PART]; K index k = 4*t + c (strided
    layout); M index m = 42*ch + 2*i + r so PSUM/OUT partition order is
    (channel, mosaic row)."""
    Wp = _packed_weights()
    mats = np.zeros((2, 3, K_PART, M_PART), np.float32)
    for (ch, r, s), Wk in Wp.items():
        for c in range(4):
            for t in range(K_ROWS):
                for i_loc in range(N_ROWS):
                    di = t - 1 - i_loc
                    if abs(di) > 1:
                        continue
                    for dj in range(-1, 2):
                        w = Wk[c, di + 1, dj + 1]
                        if w != 0:
                            mats[s, dj + 1, 4 * t + c,
                                 42 * ch + 2 * i_loc + r] = w
    return mats


_NC_CACHE = {}


def _build(mm_dtype_name="float32r", loop_iters=1, out_split=0, out_merge=1,
           in_chunks=3, halo_eng="sync"):
    """out_split: number of trailing blocks whose out-DMA rides the sync
    ring instead of scalar (ring balancing; same tensor always same ring).
    out_merge: full blocks per output DMA instruction; each group writes
    its own flat DRAM tensor outg{g} = [126, m*1024] (partition-major),
    reassembled host-side. The tail block (24) stays per-channel."""
    import concourse.bacc as bacc
    import concourse.bass as bass
    import concourse.mybir as mybir
    import concourse.tile as tile

    mm_dt = getattr(mybir.dt, mm_dtype_name)
    f32 = mybir.dt.float32

    nc = bacc.Bacc("TRN2")
    x_dt = mm_dt if mm_dtype_name == "float32r" else f32
    x = nc.dram_tensor("x", [4, H, W], x_dt, kind="ExternalInput")
    if out_merge == 1:
        outs_per_block = [
            nc.dram_tensor(f"out{b}", [3, 2 * min(N_ROWS, H - b * N_ROWS), 2 * W],
                           f32, kind="ExternalOutput")
            for b in range(N_BLOCKS)
        ]
    else:
        # groups of full blocks 0..23, tail block 24 separate
        groups = [(g * out_merge, min(out_merge, 24 - g * out_merge))
                  for g in range((24 + out_merge - 1) // out_merge)]
        outs_groups = [
            nc.dram_tensor(f"outg{g}", [M_PART, m * 2 * W], f32,
                           kind="ExternalOutput")
            for g, (b0, m) in enumerate(groups)
        ]
        out_tail = nc.dram_tensor("out24", [3, 2 * (H - 24 * N_ROWS), 2 * W],
                                  f32, kind="ExternalOutput")

    mats = _lhsT_matrices()  # [2, 3, 92, 126]
    wflat = np.concatenate([mats[s, d] for s in range(2) for d in range(3)], axis=1)
    wtens = nc.inline_tensor(wflat.copy(), name="wconst")
    mset = (lambda a: a.bitcast(f32)) if mm_dtype_name == "float32r" else (lambda a: a)

    with tile.TileContext(nc) as tc:
        with (
            tc.tile_pool(name="wpool", bufs=1) as wpool,
            tc.tile_pool(name="inpool", bufs=2) as inpool,
            tc.tile_pool(name="psum", bufs=8, space="PSUM") as psum_pool,
            tc.tile_pool(name="outpool", bufs=6) as outpool,
        ):
            w_sb = wpool.tile([K_PART, 6 * M_PART], mm_dt)
            nc.sync.dma_start(out=w_sb[:], in_=wtens[:].bitcast(mm_dt))

            # Two persistent input slots; zero both once. DMAs rewrite
            # their regions every phase; halo columns / edge zeros are
            # regions the DMAs never touch, so they stay zero.
            slots = []
            for _ in range(2):
                t = inpool.tile([K_PART, N_BLOCKS * WP], mm_dt, tag="big")
                nc.gpsimd.memset(mset(t[:]), 0.0)
                slots.append(t)

            def load(big):
                v = big.rearrange("p (b w) -> p b w", b=N_BLOCKS)
                # main pass: cols 1..23 <- rows 21b-1+t (t<21), p = 4t+c
                # split into in_chunks block-ranges per channel to cap the
                # per-engine packet size (big packets block out-ring drains)
                bounds = [1 + (23 * i) // in_chunks for i in range(in_chunks + 1)]
                for c in range(4):
                    for i in range(in_chunks):
                        b0, b1 = bounds[i], bounds[i + 1]
                        dram = bass.AP(
                            x, c * H * W + (b0 * N_ROWS - 1) * W,
                            [[W, N_ROWS], [N_ROWS * W, b1 - b0], [1, W]])
                        nc.sync.dma_start(
                            out=v[c: c + 4 * N_ROWS: 4, b0:b1, 1:513], in_=dram)
                # edge block 0: rows 0..20 -> col 0 at t=1..21
                for c in range(4):
                    nc.sync.dma_start(
                        out=v[4 + c: 4 + c + 4 * N_ROWS: 4, 0, 1:513],
                        in_=x[c, 0:N_ROWS, :])
                # edge block 24: rows 503..511 -> col 24 at t=0..8
                for c in range(4):
                    nc.sync.dma_start(
                        out=v[c: c + 36: 4, 24, 1:513],
                        in_=x[c, 503:512, :])
                # halo: cols 0..23 t=21,22  <-  cols 1..24 t=0,1
                heng = {"sync": nc.sync, "scalar": nc.scalar}[halo_eng]
                heng.dma_start(out=v[84:92, 0:24, 1:513],
                               in_=v[0:8, 1:25, 1:513])

            def compute(big):
                o_t = None
                for b in range(N_BLOCKS):
                    i0 = b * N_ROWS
                    nrows = min(N_ROWS, H - i0)
                    tail = nrows < N_ROWS
                    if out_merge == 1 or tail:
                        o_t = outpool.tile([M_PART, 2 * W], f32, tag="osm")
                        col = 0
                    else:
                        gi, off = divmod(b, out_merge)
                        if off == 0:
                            m_g = min(out_merge, 24 - gi * out_merge)
                            o_t = outpool.tile([M_PART, m_g * 2 * W], f32,
                                               tag="obig")
                        col = off * 2 * W
                    ps = []
                    for s in range(2):
                        p = psum_pool.tile([M_PART, W], f32)
                        for dj in (-1, 0, 1):
                            q = 3 * s + (dj + 1)
                            nc.tensor.matmul(
                                p[:],
                                w_sb[:, M_PART * q: M_PART * (q + 1)],
                                big[:, WP * b + 1 + dj: WP * b + 1 + dj + W],
                                start=(dj == -1),
                                stop=(dj == 1),
                            )
                        ps.append(p)
                    nc.vector.tensor_copy(out=o_t[:, col: col + 2 * W: 2],
                                          in_=ps[0][:])
                    nc.scalar.copy(out=o_t[:, col + 1: col + 2 * W: 2],
                                   in_=ps[1][:])

                    eng = nc.sync if b >= N_BLOCKS - out_split else nc.scalar
                    if out_merge == 1:
                        ob = outs_per_block[b]
                        if not tail:
                            eng.dma_start(out=ob[:, :, :], in_=o_t[:])
                        else:
                            for ch in range(3):
                                eng.dma_start(
                                    out=ob[ch, :, :],
                                    in_=o_t[2 * N_ROWS * ch: 2 * N_ROWS * ch + 2 * nrows, :])
                    elif tail:
                        for ch in range(3):
                            eng.dma_start(
                                out=out_tail[ch, :, :],
                                in_=o_t[2 * N_ROWS * ch: 2 * N_ROWS * ch + 2 * nrows, :])
                    elif (b + 1) % out_merge == 0 or b == 23:
                        gi = b // out_merge
                        eng.dma_start(out=outs_groups[gi][:, :], in_=o_t[:])

            loop_cm = tc.For_i(0, loop_iters, 1) if loop_iters > 1 else contextlib.nullcontext()
            with loop_cm:
                load(slots[0])
                compute(slots[1])
                load(slots[1])
                compute(slots[0])
            # epilogue: the last load went to slots[1]; compute it so a
            # single-shot run produces the real result as the last write.
            compute(slots[1])
    nc.compile()
    return nc


def _get_nc(mm_dtype_name="float32r", loop_iters=1, out_split=0, out_merge=1,
            in_chunks=3, halo_eng="sync"):
    key = (mm_dtype_name, loop_iters, out_split, out_merge, in_chunks, halo_eng)
    if key not in _NC_CACHE:
        _NC_CACHE[key] = _build(mm_dtype_name, loop_iters, out_split, out_merge,
                                in_chunks, halo_eng)
    return _NC_CACHE[key]


OUT_MERGE = 1


def kernel(x: np.ndarray, mm_dtype_name: str = "float32r",
           out_merge: int = OUT_MERGE, **run_kwargs) -> np.ndarray:
    from concourse.bass_utils import run_bass_kernel_spmd

    x = np.ascontiguousarray(np.asarray(x), dtype=np.float32)
    assert x.shape == (N_CORES, 4, H, W), x.shape
    nc = _get_nc(mm_dtype_name, out_merge=out_merge)
    in_maps = [{"x": x[b]} for b in range(N_CORES)]
    res = run_bass_kernel_spmd(nc, in_maps, core_ids=list(range(N_CORES)), **run_kwargs)

    def gather(r):
        if out_merge == 1:
            return np.concatenate([r[f"out{b}"] for b in range(N_BLOCKS)], axis=1)
        full = np.empty((3, 2 * H, 2 * W), np.float32)
        n_groups = (24 + out_merge - 1) // out_merge
        for g in range(n_groups):
            b0 = g * out_merge
            m = min(out_merge, 24 - b0)
            # [126, m*1024] -> (ch, row2, blk, w)
            a = r[f"outg{g}"].reshape(3, 2 * N_ROWS, m, 2 * W)
            a = a.transpose(0, 2, 1, 3).reshape(3, m * 2 * N_ROWS, 2 * W)
            full[:, 2 * N_ROWS * b0: 2 * N_ROWS * (b0 + m), :] = a
        full[:, 2 * N_ROWS * 24:, :] = r["out24"]
        return full
    return np.stack([gather(r) for r in res.results], axis=0)


if __name__ == "__main__":
    x = np.random.rand(N_CORES, 4, H, W).astype(np.float32)
    y = kernel(x)
    print("out", y.shape, y.dtype, float(y.sum()))



# revision 2
# speedup vs baseline: 1.3501x; 1.3501x over previous
"""Malvar demosaic on Trainium2 (Bass/Tile), 8-core data parallel — v3.

All-bf16 dataflow (gate is 2e-2; bf16 end-to-end costs ~2e-3 L2):
  - host pre-permutes x [4,H,W] f32 -> row-interleaved [4H, W] bf16
    (row 4i+c = x[c,i]), so each block's 92 SBUF partitions map to 92
    CONSECUTIVE DRAM rows -> the whole input (row halo included, read
    twice from HBM) loads in 3 DMA instructions on the sync ring.
  - matmuls: lhsT bf16 [92,126] x rhs bf16 [92,512] -> PSUM f32
    (TRN2 requires f32 PSUM). 6 matmuls per 21-row block (2 psum
    tiles x 3 column shifts), 150 per image.
  - PSUM -> SBUF interleave copies cast f32->bf16 (vector: even cols,
    scalar: odd cols).
  - output: blocks grouped 4-wide into [126, 4096] bf16 DRAM tensors
    (6 groups + 1 tail) = 7 DMA instructions, split across the two
    HWDGE rings (sync/scalar) to balance bytes; host reassembles and
    casts back to f32.

Software pipelining: For_i body = load(A); compute(B); load(B);
compute(A) (2 images per iteration), plus an epilogue compute so a
single-shot run's last write is the real result.

Layout: big tile [92, 25*514] bf16; block b at free cols [514b,
514b+514) with zero halo columns at offsets 0 and 513; partition
p = 4t + c holds input row 21b-1+t of channel c (t in [0,23)).
"""
import contextlib

import ml_dtypes
import numpy as np

H, W = 512, 512
N_CORES = 8
N_ROWS = 21            # output packed rows per block
K_ROWS = N_ROWS + 2    # input rows incl halo
K_PART = 4 * K_ROWS    # 92
M_PART = 6 * N_ROWS    # 126
N_BLOCKS = (H + N_ROWS - 1) // N_ROWS  # 25
WP = W + 2             # per-block column pitch
OUT_GROUP = 4          # full blocks per output DRAM tensor
N_GROUPS = 24 // OUT_GROUP  # 6 (block 24 is the tail)

_G_AT_R = np.array([[0,0,-1,0,0],[0,0,2,0,0],[-1,2,4,2,-1],[0,0,2,0,0],[0,0,-1,0,0]], np.float32) / 8
_R_AT_G1 = np.array([[0,0,0.5,0,0],[0,-1,0,-1,0],[-1,4,5,4,-1],[0,-1,0,-1,0],[0,0,0.5,0,0]], np.float32) / 8
_R_AT_G2 = np.array([[0,0,-1,0,0],[0,-1,4,-1,0],[0.5,0,5,0,0.5],[0,-1,4,-1,0],[0,0,-1,0,0]], np.float32) / 8
_R_AT_B = np.array([[0,0,-1.5,0,0],[0,2,0,2,0],[-1.5,0,6,0,-1.5],[0,2,0,2,0],[0,0,-1.5,0,0]], np.float32) / 8

PLANES = {
    (0, 0, 0): ('conv', _R_AT_B),
    (0, 0, 1): ('conv', _R_AT_G2),
    (0, 1, 0): ('conv', _R_AT_G1),
    (0, 1, 1): ('id', 2),
    (1, 0, 0): ('conv', _G_AT_R),
    (1, 0, 1): ('id', 0),
    (1, 1, 0): ('id', 3),
    (1, 1, 1): ('conv', _G_AT_R),
    (2, 0, 0): ('id', 1),
    (2, 0, 1): ('conv', _R_AT_G1),
    (2, 1, 0): ('conv', _R_AT_G2),
    (2, 1, 1): ('conv', _R_AT_B),
}


def _packed_weights():
    out = {}
    for (ch, r, s), (kind, val) in PLANES.items():
        Wk = np.zeros((4, 3, 3), np.float32)
        if kind == 'id':
            Wk[val, 1, 1] = 1.0
        else:
            for u in range(-2, 3):
                for v in range(-2, 3):
                    w = val[u + 2, v + 2]
                    if w == 0:
                        continue
                    rc = (r + u) % 2
                    di = (r + u - rc) // 2
                    sc = (s + v) % 2
                    dj = (s + v - sc) // 2
                    Wk[2 * rc + sc, di + 1, dj + 1] += w
        out[(ch, r, s)] = Wk
    return out


def _lhsT_matrices():
    """lhsT[s][dj] as [K_PART, M_PART]; K index k = 4*t + c (strided
    layout); M index m = 42*ch + 2*i + r so PSUM/OUT partition order is
    (channel, mosaic row)."""
    Wp = _packed_weights()
    mats = np.zeros((2, 3, K_PART, M_PART), np.float32)
    for (ch, r, s), Wk in Wp.items():
        for c in range(4):
            for t in range(K_ROWS):
                for i_loc in range(N_ROWS):
                    di = t - 1 - i_loc
                    if abs(di) > 1:
                        continue
                    for dj in range(-1, 2):
                        w = Wk[c, di + 1, dj + 1]
                        if w != 0:
                            mats[s, dj + 1, 4 * t + c,
                                 42 * ch + 2 * i_loc + r] = w
    return mats


_NC_CACHE = {}


def _build(loop_iters=1, in_chunks=1, out_sync_groups=2):
    import concourse.bacc as bacc
    import concourse.bass as bass
    import concourse.mybir as mybir
    import concourse.tile as tile

    bf16 = mybir.dt.bfloat16
    f32 = mybir.dt.float32

    nc = bacc.Bacc("TRN2")
    x = nc.dram_tensor("x", [4 * H, W], bf16, kind="ExternalInput")
    outs_groups = [
        nc.dram_tensor(f"outg{g}", [M_PART, OUT_GROUP * 2 * W], bf16,
                       kind="ExternalOutput")
        for g in range(N_GROUPS)
    ]
    out_tail = nc.dram_tensor("out24", [M_PART, 2 * W], bf16,
                              kind="ExternalOutput")

    mats = _lhsT_matrices()  # [2, 3, 92, 126]
    wflat = np.concatenate([mats[s, d] for s in range(2) for d in range(3)],
                           axis=1).astype(ml_dtypes.bfloat16)
    wtens = nc.inline_tensor(wflat.copy(), name="wconst")

    with tile.TileContext(nc) as tc:
        with (
            tc.tile_pool(name="wpool", bufs=1) as wpool,
            tc.tile_pool(name="inpool", bufs=2) as inpool,
            tc.tile_pool(name="psum", bufs=8, space="PSUM") as psum_pool,
            tc.tile_pool(name="outpool", bufs=4) as outpool,
        ):
            w_sb = wpool.tile([K_PART, 6 * M_PART], bf16)
            nc.sync.dma_start(out=w_sb[:], in_=wtens[:])

            # Two persistent input slots; zero both once. DMAs rewrite
            # their regions every phase; halo columns / edge zeros are
            # regions the DMAs never touch, so they stay zero.
            slots = []
            for _ in range(2):
                t = inpool.tile([K_PART, N_BLOCKS * WP], bf16, tag="big")
                nc.gpsimd.memset(t[:], 0.0)
                slots.append(t)

            def load(big):
                v = big.rearrange("p (b w) -> p b w", b=N_BLOCKS)
                # block 0: p = 4..91 <- rows 0..87 (row -1 stays zero)
                nc.sync.dma_start(
                    out=v[4:92, 0, 1:513],
                    in_=bass.AP(x, 0, [[W, 88], [1, W]]))
                # blocks 1..23: partition p <- row 84b - 4 + p (halo rows
                # re-read from HBM; no SBUF->SBUF copy needed)
                bounds = [1 + (23 * i) // in_chunks for i in range(in_chunks + 1)]
                for i in range(in_chunks):
                    b0, b1 = bounds[i], bounds[i + 1]
                    nc.sync.dma_start(
                        out=v[0:92, b0:b1, 1:513],
                        in_=bass.AP(x, (84 * b0 - 4) * W,
                                    [[W, 92], [84 * W, b1 - b0], [1, W]]))
                # block 24: p = 0..35 <- rows 2012..2047 (rest stays zero)
                nc.sync.dma_start(
                    out=v[0:36, 24, 1:513],
                    in_=bass.AP(x, 2012 * W, [[W, 36], [1, W]]))

            def do_block(big, b, o_t, col):
                ps = []
                for s in range(2):
                    p = psum_pool.tile([M_PART, W], f32)
                    for dj in (-1, 0, 1):
                        q = 3 * s + (dj + 1)
                        nc.tensor.matmul(
                            p[:],
                            w_sb[:, M_PART * q: M_PART * (q + 1)],
                            big[:, WP * b + 1 + dj: WP * b + 1 + dj + W],
                            start=(dj == -1),
                            stop=(dj == 1),
                        )
                    ps.append(p)
                nc.vector.tensor_copy(out=o_t[:, col: col + 2 * W: 2],
                                      in_=ps[0][:])
                nc.scalar.copy(out=o_t[:, col + 1: col + 2 * W: 2],
                               in_=ps[1][:])

            def compute(big):
                for g in range(N_GROUPS):
                    o_t = outpool.tile([M_PART, OUT_GROUP * 2 * W], bf16,
                                       tag="obig")
                    for off in range(OUT_GROUP):
                        do_block(big, OUT_GROUP * g + off, o_t, off * 2 * W)
                    eng = nc.sync if g >= N_GROUPS - out_sync_groups else nc.scalar
                    eng.dma_start(out=outs_groups[g][:, :], in_=o_t[:])
                o_t = outpool.tile([M_PART, 2 * W], bf16, tag="otail")
                do_block(big, 24, o_t, 0)
                nc.scalar.dma_start(out=out_tail[:, :], in_=o_t[:])

            loop_cm = tc.For_i(0, loop_iters, 1) if loop_iters > 1 else contextlib.nullcontext()
            with loop_cm:
                load(slots[0])
                compute(slots[1])
                load(slots[1])
                compute(slots[0])
            # epilogue: the last load went to slots[1]; compute it so a
            # single-shot run produces the real result as the last write.
            compute(slots[1])
    nc.compile()
    return nc


def _get_nc(loop_iters=1, in_chunks=1, out_sync_groups=2):
    key = (loop_iters, in_chunks, out_sync_groups)
    if key not in _NC_CACHE:
        _NC_CACHE[key] = _build(loop_iters, in_chunks, out_sync_groups)
    return _NC_CACHE[key]


def kernel(x: np.ndarray, **run_kwargs) -> np.ndarray:
    from concourse.bass_utils import run_bass_kernel_spmd

    x = np.asarray(x)
    assert x.shape == (N_CORES, 4, H, W), x.shape
    # row-interleave channels: row 4i+c = x[core, c, i] -> [4H, W] bf16
    xr = np.ascontiguousarray(
        x.transpose(0, 2, 1, 3)).reshape(N_CORES, 4 * H, W)
    xr = xr.astype(ml_dtypes.bfloat16)
    nc = _get_nc()
    in_maps = [{"x": xr[b]} for b in range(N_CORES)]
    res = run_bass_kernel_spmd(nc, in_maps, core_ids=list(range(N_CORES)),
                               **run_kwargs)

    def gather(r):
        full = np.empty((3, 2 * H, 2 * W), np.float32)
        for g in range(N_GROUPS):
            # [126, 4*1024] -> (ch, row2, blk, w) -> rows 168g..168(g+1)
            a = np.asarray(r[f"outg{g}"]).astype(np.float32)
            a = a.reshape(3, 2 * N_ROWS, OUT_GROUP, 2 * W)
            a = a.transpose(0, 2, 1, 3).reshape(3, OUT_GROUP * 2 * N_ROWS, 2 * W)
            full[:, 168 * g: 168 * (g + 1), :] = a
        tail = np.asarray(r["out24"]).astype(np.float32).reshape(3, 2 * N_ROWS, 2 * W)
        full[:, 2 * N_ROWS * 24:, :] = tail[:, : 2 * (H - 24 * N_ROWS), :]
        return full

    return np.stack([gather(r) for r in res.results], axis=0)


if __name__ == "__main__":
    x = np.random.rand(N_CORES, 4, H, W).astype(np.float32)
    y = kernel(x)
    print("out", y.shape, y.dtype, float(y.sum()))
